# revision 11
# baseline (speedup 1.0000x reference)
"""MoE-routing LoRA linear for Trainium2, SPMD over 8 NeuronCores.

out = x @ base_w.T + base_b + 2.0 * lora_out, where lora_out routes each
token through its top-2 (of 8) LoRA experts with renormalized softmax gates.

Strategy: data-parallel over tokens (1024 tokens/core), weights replicated.
All heavy FLOPs are fp32r matmuls (full PE rate). The per-expert LoRA is
algebraically dense: h = x @ A_cat.T ([T,256]); gated hg = h * gates[e];
lora_out = hg @ W2 ([256,4096]) which is fused into the base matmul as two
extra contraction chunks.
"""

import numpy as np

P = 128
B, S, D, O, E, R = 4, 2048, 4096, 4096, 8, 32
T = B * S            # 8192 tokens
NCORES = 8
TC = T // NCORES     # 1024 tokens per core
TT = TC // P         # 8 token tiles per core
DC = D // P          # 32 contraction chunks for x
ER = E * R           # 256
HC = ER // P         # 2 contraction chunks for hg
KC = DC + HC         # 34 total contraction chunks in the fused matmul
ON = 512             # output tile width
OT = O // ON         # 8 output tiles
SCALING = 64.0 / 32.0

TRACE = False        # test harness sets kernel.TRACE = True for profiling
LAST_RESULT = None   # BassKernelResults of the last run (for exec_time_ns)

_compiled = None


def _build():
    import concourse.mybir as mybir
    import concourse.tile as tile
    from concourse import bacc
    from concourse.masks import make_identity

    f32 = mybir.dt.float32
    f32r = mybir.dt.float32r
    X = mybir.AxisListType.X
    mult = mybir.AluOpType.mult
    is_ge = mybir.AluOpType.is_ge
    Exp = mybir.ActivationFunctionType.Exp

    nc = bacc.Bacc("TRN2", target_bir_lowering=False, debug=False,
                   num_devices=NCORES)

    xs = nc.dram_tensor("xs", [TC, D], f32, kind="ExternalInput").ap()
    # fp32r-declared DRAM weights: DMA straight into fp32r SBUF tiles is
    # accepted by the BIR verifier and bit-identical to a DVE rounding pass
    # (verified empirically on HW).
    wbig = nc.dram_tensor("wbig", [KC * P, O], f32r, kind="ExternalInput").ap()
    wcat = nc.dram_tensor("wcat", [D, ER], f32r, kind="ExternalInput").ap()
    # router weights kept fp32: exact logits so top-2 selection matches the
    # fp32 reference (fp32r noise flips near-tied experts otherwise)
    wrouter = nc.dram_tensor("wrouter", [D, E], f32, kind="ExternalInput").ap()
    bias = nc.dram_tensor("bias", [P, O], f32, kind="ExternalInput").ap()
    cbias = nc.dram_tensor("cbias", [P, E], f32, kind="ExternalInput").ap()
    out = nc.dram_tensor("out", [TC, O], f32, kind="ExternalOutput").ap()

    NCAT = ER  # 256

    with tile.TileContext(nc) as tc:
        with (
            tc.tile_pool(name="persist", bufs=1) as persist,
            tc.tile_pool(name="consts", bufs=1) as consts,
        ):
            xT = persist.tile([P, DC, TC], f32r)     # x transposed, rounded
            hgT = persist.tile([P, HC, TC], f32r)    # gated h transposed
            ident = consts.tile([P, P], f32)
            make_identity(nc, ident[:])
            bias_sb = consts.tile([P, O], f32)
            nc.sync.dma_start(bias_sb[:], bias)
            cbias_sb = consts.tile([P, E], f32)
            nc.sync.dma_start(cbias_sb[:], cbias)
            wrouter_sb = consts.tile([P, DC, E], f32)
            nc.sync.dma_start(
                wrouter_sb[:], wrouter.rearrange("(kc p) n -> p kc n", p=P))
            negbig = consts.tile([P, E], f32)
            nc.vector.memset(negbig[:], -1e30)

            # ---------------- Phase 1: transpose x, router, gates, hgT ----
            with (
                tc.tile_pool(name="wcat_pool", bufs=1) as wcat_pool,
                tc.tile_pool(name="p1", bufs=3) as p1,
                tc.tile_pool(name="gates_pool", bufs=2) as gp,
                tc.tile_pool(name="ph_psum", bufs=2, space="PSUM") as php,
                tc.tile_pool(name="pr_psum", bufs=2, space="PSUM") as prp,
                tc.tile_pool(name="tr_psum", bufs=4, space="PSUM") as ptp,
            ):
                wcat_sb = wcat_pool.tile([P, DC, NCAT], f32r)
                nc.sync.dma_start(
                    wcat_sb[:], wcat.rearrange("(kc p) n -> p kc n", p=P))

                for tt in range(TT):
                    ts = slice(tt * P, (tt + 1) * P)
                    # transpose x tile [128, 4096] -> xT[:, :, ts]
                    for dc4 in range(4):
                        xc = p1.tile([P, 1024], f32, tag="xc")
                        nc.sync.dma_start(
                            xc[:], xs[ts, dc4 * 1024:(dc4 + 1) * 1024])
                        for j in range(8):
                            pt = ptp.tile([P, P], f32, tag="pt")
                            nc.tensor.transpose(
                                pt[:], xc[:, j * P:(j + 1) * P], ident[:])
                            nc.vector.tensor_copy(
                                xT[:, dc4 * 8 + j, ts], pt[:])
                    # loraA: psum_h[t, 256] = sum_k xT.T @ wcat  (fp32r)
                    ph = php.tile([P, NCAT], f32, tag="ph")
                    for kc in range(DC):
                        nc.tensor.matmul(ph[:], xT[:, kc, ts],
                                         wcat_sb[:, kc, :],
                                         start=(kc == 0), stop=(kc == DC - 1))
                    # router logits in exact fp32 (tiny: 8 cols)
                    pr = prp.tile([P, E], f32, tag="pr")
                    for kc in range(DC):
                        nc.tensor.matmul(pr[:], xT[:, kc, ts].bitcast(f32),
                                         wrouter_sb[:, kc, :],
                                         start=(kc == 0), stop=(kc == DC - 1))
                    lg_sb = gp.tile([P, E], f32, tag="lgsb")
                    nc.vector.tensor_add(lg_sb[:], pr[:], cbias_sb[:])
                    lg = lg_sb[:]
                    # top-2 renormalized softmax gates (x SCALING)
                    m1 = gp.tile([P, 1], f32, tag="m1")
                    nc.vector.reduce_max(m1[:], lg, axis=X)
                    negm1 = gp.tile([P, 1], f32, tag="negm1")
                    nc.scalar.mul(negm1[:], m1[:], -1.0)
                    e_sb = gp.tile([P, E], f32, tag="esb")
                    nc.scalar.activation(e_sb[:], lg, Exp, bias=negm1[:])
                    t1 = gp.tile([P, E], f32, tag="t1")
                    nc.vector.scalar_tensor_tensor(
                        t1[:], lg, m1[:], negbig[:], is_ge, mult)
                    masked = gp.tile([P, E], f32, tag="masked")
                    nc.vector.tensor_add(masked[:], lg, t1[:])
                    m2 = gp.tile([P, 1], f32, tag="m2")
                    nc.vector.reduce_max(m2[:], masked[:], axis=X)
                    g_sb = gp.tile([P, E], f32, tag="gsb")
                    dsum = gp.tile([P, 1], f32, tag="dsum")
                    nc.vector.scalar_tensor_tensor(
                        g_sb[:], lg, m2[:], e_sb[:], is_ge, mult,
                        accum_out=dsum[:])
                    dhalf = gp.tile([P, 1], f32, tag="dhalf")
                    nc.scalar.mul(dhalf[:], dsum[:], 1.0 / SCALING)
                    rinv = gp.tile([P, 1], f32, tag="rinv")
                    nc.vector.reciprocal(rinv[:], dhalf[:])
                    gates = gp.tile([P, E], f32, tag="gates")
                    nc.vector.tensor_scalar_mul(gates[:], g_sb[:], rinv[:])
                    # hg = h * gates (broadcast over r), straight from PSUM
                    hg = gp.tile([P, ER], f32, tag="hg")
                    nc.vector.tensor_tensor(
                        hg[:].rearrange("p (e r) -> p e r", e=E),
                        ph[:].rearrange("p (e r) -> p e r", e=E),
                        gates[:, :, None].to_broadcast([P, E, R]),
                        mult)
                    for j in range(HC):
                        pt = ptp.tile([P, P], f32, tag="pt")
                        nc.tensor.transpose(
                            pt[:], hg[:, j * P:(j + 1) * P], ident[:])
                        nc.vector.tensor_copy(hgT[:, j, ts], pt[:])

            # ---------------- Phase 2: fused [xT; hgT] @ wbig + bias ------
            with (
                tc.tile_pool(name="wstream", bufs=4) as wst,
                tc.tile_pool(name="outp", bufs=4) as outp,
                tc.tile_pool(name="po_psum", bufs=8, space="PSUM") as pop,
            ):
                KP = KC // 2  # 17 chunk-pairs
                for ot in range(OT):
                    osl = slice(ot * ON, (ot + 1) * ON)
                    ptiles = [pop.tile([P, ON], f32, tag="po",
                                       name=f"po_{ot}_{tt}")
                              for tt in range(TT)]
                    for kp in range(KP):
                        wt = wst.tile([P, 2, ON], f32r, tag="w32")
                        nc.sync.dma_start(
                            wt[:],
                            wbig[kp * 2 * P:(kp + 1) * 2 * P, osl]
                            .rearrange("(c p) n -> p c n", p=P))
                        for c in range(2):
                            kc = kp * 2 + c
                            for tt in range(TT):
                                ts = slice(tt * P, (tt + 1) * P)
                                lhsT = (xT[:, kc, ts] if kc < DC
                                        else hgT[:, kc - DC, ts])
                                nc.tensor.matmul(
                                    ptiles[tt][:], lhsT, wt[:, c, :],
                                    start=(kc == 0), stop=(kc == KC - 1))
                    for tt in range(TT):
                        ts = slice(tt * P, (tt + 1) * P)
                        osb = outp.tile([P, ON], f32, tag="osb")
                        nc.vector.tensor_add(
                            osb[:], ptiles[tt][:], bias_sb[:, osl])
                        nc.sync.dma_start(out[ts, osl], osb[:])

    nc.compile()
    return nc


def _get_compiled():
    global _compiled
    if _compiled is None:
        _compiled = _build()
    return _compiled


def kernel(**inputs):
    global LAST_RESULT
    from concourse.bass_utils import run_bass_kernel_spmd

    x = np.ascontiguousarray(np.asarray(inputs["x"], dtype=np.float32))
    base_w = np.asarray(inputs["base_w"], dtype=np.float32)
    base_b = np.asarray(inputs["base_b"], dtype=np.float32)
    router_w = np.asarray(inputs["router_w"], dtype=np.float32)
    router_b = np.asarray(inputs["router_b"], dtype=np.float32)
    lora_a = np.asarray(inputs["lora_a"], dtype=np.float32)
    lora_b = np.asarray(inputs["lora_b"], dtype=np.float32)
    top_k = int(np.asarray(inputs.get("top_k", 2)))
    assert top_k == 2, "kernel is specialized for top_k=2"

    xt = x.reshape(T, D)
    # wbig rows: base_w.T (4096) then W2 (256) with W2[e*R+r, o] = lora_b[e,o,r]
    w2 = np.ascontiguousarray(
        lora_b.transpose(0, 2, 1).reshape(ER, O).astype(np.float32))
    wbig = np.ascontiguousarray(
        np.concatenate([base_w.T, w2], axis=0).astype(np.float32))
    # wcat: A_cat columns [d, er]; router weights separate (fp32-exact path)
    acat = lora_a.reshape(ER, D)  # [er, d]
    wcat = np.ascontiguousarray(acat.T.astype(np.float32))
    wrouter = np.ascontiguousarray(router_w.T.astype(np.float32))
    bias_full = np.ascontiguousarray(
        np.broadcast_to(base_b, (P, O)).astype(np.float32))
    cbias_full = np.ascontiguousarray(
        np.broadcast_to(router_b.astype(np.float32), (P, E)))

    nc = _get_compiled()
    in_maps = [
        {"xs": np.ascontiguousarray(xt[c * TC:(c + 1) * TC]),
         "wbig": wbig, "wcat": wcat, "wrouter": wrouter,
         "bias": bias_full, "cbias": cbias_full}
        for c in range(NCORES)
    ]
    res = run_bass_kernel_spmd(nc, in_maps, core_ids=list(range(NCORES)),
                               trace=TRACE)
    LAST_RESULT = res
    outp = np.concatenate(
        [res.results[c]["out"] for c in range(NCORES)], axis=0)
    return outp.reshape(B, S, O).astype(np.float32)
